# revision 25
# baseline (speedup 1.0000x reference)
"""Trainium2 Bass kernel: 4-layer pose-temporal transformer encoder.

kernel(**inputs) takes FULL unsharded fp32 inputs, returns FULL (16,512,1024)
fp32 output.  Data-parallel over batch across 8 NeuronCores (2 batch elements
per core, no collectives).

Per-core: feature-major fp32 residual x[E(part), tokens].  QKV/out
projections and A@V run in fp8(e4m3) with DoubleRow perf mode (per-tensor
host-side weight scaling, descale factors streamed as a DRAM param so the
NEFF is value-independent); MLP and attention scores stay bf16 with fp32
PSUM accumulation (fp8 anywhere in the MLP costs ~2.1% output error per
quantization point -- over the 2e-2 budget).  The relative-position bias
is precomputed on the host as exp(bias) in a (L,H,T,T) bf16 table and
folded in AFTER the exp: ACT emits 8*exp(s) to bf16, Pool multiplies by
exp(bias) into fp8 (keeps the PE out of the bias entirely); the
ones-column of V yields the softmax denominators, applied as a
paired-head broadcast matmul + multiply.  LayerNorm statistics use
matmul partition-reductions with copies/squares spread across
DVE/ACT/Pool and a fused row chain; stats for all chunks are emitted
before applies so engine FIFOs never block chunk c+1 behind chunk c.
Weight/bias streams are double-buffered and spread over both HW DGE
queues (SP + ACT) plus the Pool SWDGE.
"""

import numpy as np
import ml_dtypes
from contextlib import ExitStack

import concourse.bass as bass
import concourse.tile as tile
from concourse import bacc, mybir
from concourse.bass_utils import run_bass_kernel_spmd

F32 = mybir.dt.float32
F32R = mybir.dt.float32r
FP8 = mybir.dt.float8e4
LN16 = 2.0794415416798357  # ln(8): pts8 = 8*exp(s), max ~205 < 448
BF16 = mybir.dt.bfloat16
AF = mybir.ActivationFunctionType
ALU = mybir.AluOpType
P = 128

FULL = dict(BL=2, T=512, E=1024, H=16, FF=4096, L=4)
N_CORES = 8
EPS = 1e-5
MAX_OFFSET = 0.5


def build_nc(cfg, flags=frozenset()):
    BL, T, E, H, FF, L = cfg["BL"], cfg["T"], cfg["E"], cfg["H"], cfg["FF"], cfg["L"]
    HD = E // H
    EO = E // P
    FO = FF // P
    TOK = BL * T
    CH = min(512, T)
    NCH = TOK // CH
    TQ = T // P
    HPT = max(1, P // HD)
    WS = min(512, E)
    WQK = min(256, E)       # q/k strip width (SBUF pressure)

    nc = bacc.Bacc(None, target_bir_lowering=False,
                   debug=bool(cfg.get("debug", False)))

    x_d = nc.declare_dram_parameter("x_fm", [E, TOK], F32, False)
    wq_d = nc.declare_dram_parameter("wq", [L, E, E], FP8, False)
    wk_d = nc.declare_dram_parameter("wk", [L, E, E], FP8, False)
    wv_d = nc.declare_dram_parameter("wv", [L, E, E], FP8, False)
    wo_d = nc.declare_dram_parameter("wo", [L, E, E], FP8, False)
    scl_d = nc.declare_dram_parameter("scl", [P, L * 6], F32, False)
    w1_d = nc.declare_dram_parameter("w1", [L, E, FF], BF16, False)
    w2_d = nc.declare_dram_parameter("w2", [L, FF, E], BF16, False)
    bm_d = nc.declare_dram_parameter("biasmat", [L, H, T, T], BF16, False)
    extra = {}
    for nm, shp in [("bq", [L, E]), ("bk", [L, E]), ("bv", [L, E]),
                    ("bo", [L, E]), ("b1", [L, FF]), ("b2", [L, E]),
                    ("ln1_g", [L, E]), ("ln1_b", [L, E]),
                    ("ln2_g", [L, E]), ("ln2_b", [L, E])]:
        key = nm.split("_")[0] if nm.startswith("ln") else nm
        if key in flags:
            extra[nm] = nc.declare_dram_parameter(nm, shp, F32, False)
    out_d = nc.declare_dram_parameter("out_fm", [E, TOK], F32, True)

    with tile.TileContext(nc) as tc, ExitStack() as ctx:
        const = ctx.enter_context(tc.tile_pool(name="const", bufs=1))
        resid = ctx.enter_context(tc.tile_pool(name="resid", bufs=1))
        hpool = ctx.enter_context(tc.tile_pool(name="hpool", bufs=2))
        qkpool = ctx.enter_context(tc.tile_pool(name="qkpool", bufs=2))
        vhpool = ctx.enter_context(tc.tile_pool(name="vhpool", bufs=1))
        wpool = ctx.enter_context(tc.tile_pool(name="wpool", bufs=2))
        w2pool = ctx.enter_context(tc.tile_pool(name="w2pool", bufs=3))
        lnpool = ctx.enter_context(tc.tile_pool(name="lnpool", bufs=2))
        rowpool = ctx.enter_context(tc.tile_pool(name="rowpool", bufs=1))
        ptpool = ctx.enter_context(tc.tile_pool(name="ptpool", bufs=2))
        bpool = ctx.enter_context(tc.tile_pool(name="bpool", bufs=2))
        ps = ctx.enter_context(tc.tile_pool(name="ps", bufs=1, space="PSUM"))

        def psum(shape, name, tag, bufs):
            return ps.tile(shape, F32, name=name, tag=tag, bufs=bufs)

        ones_col = const.tile([P, 1], BF16)
        nc.vector.memset(ones_col, 1.0)
        ones_col_f = const.tile([P, 1], F32)
        nc.vector.memset(ones_col_f, 1.0)
        ones_row = const.tile([1, P], F32)
        nc.vector.memset(ones_row, 1.0)
        ones_row_bf = const.tile([1, P], BF16)
        nc.vector.memset(ones_row_bf, 1.0)
        onesA = const.tile([1, P], BF16)    # ones on cols 0..HD-1
        nc.vector.memset(onesA, 0.0)
        nc.vector.memset(onesA[0:1, 0:HD], 32.0)
        onesB = const.tile([1, P], BF16)    # ones on cols HD..127
        nc.vector.memset(onesB, 0.0)
        nc.vector.memset(onesB[0:1, HD:2 * HD], 32.0)
        eps_c = const.tile([1, 1], F32)
        nc.vector.memset(eps_c, EPS)
        eps_cq = const.tile([1, 1], F32)
        nc.vector.memset(eps_cq, EPS / 1024.0)
        ln16_c = const.tile([P, 1], F32)
        nc.vector.memset(ln16_c, LN16)
        scl_sb = const.tile([P, L * 6], F32, name="scl_sb", tag="scl_sb")
        nc.sync.dma_start(out=scl_sb, in_=scl_d[:, :])

        def load_param_cols(dram_row, n_tiles, nm):
            t = const.tile([P, n_tiles], F32, name=nm, tag=nm)
            nc.sync.dma_start(out=t, in_=dram_row.rearrange("(o p) -> p o", p=P))
            return t

        params = {}
        for l in range(L):
            for nm in ("bq", "bk", "bo", "b1", "b2"):
                if nm in extra:
                    n_t = FO if nm == "b1" else EO
                    params[(nm, l)] = load_param_cols(extra[nm][l], n_t, f"{nm}{l}")
            for nm in ("ln1_g", "ln1_b", "ln2_g", "ln2_b"):
                if nm in extra:
                    params[(nm, l)] = load_param_cols(extra[nm][l], EO, f"{nm}{l}")

        x_sb = resid.tile([P, EO, TOK], F32)

        def layernorm(g, b, q8=False):
            """LN of x_sb (feature-major); stats for ALL chunks emitted
            first (so chunk c+1 stats aren't queued behind chunk c apply),
            then the applies.  q8: emit fp8 h scaled by 32 (folded into
            rstd via sqrt((va+eps)/1024))."""
            out = hpool.tile([P, EO, TOK], FP8 if q8 else BF16,
                             name="hs", tag="hs")
            ctx2 = tc.high_priority()
            ctx2.__enter__()
            acs = []
            for c in range(NCH):
                csl = bass.ts(c, CH)
                ssum = psum([1, CH], "ssum", "sps", 3)
                ssq = psum([1, CH], "ssq", "sps", 3)
                # copies+squares split over DVE / ACT / Pool; stats MMs
                # interleave per eo
                for eo in range(EO):
                    xb = lnpool.tile([P, CH], BF16, name="xbc", tag="xbc",
                                     bufs=2)
                    sq = lnpool.tile([P, CH], BF16, name="sqc", tag="sqc",
                                     bufs=2)
                    xs = x_sb[:, eo, csl]
                    if eo < 4:
                        nc.vector.tensor_copy(out=xb, in_=xs)
                        nc.vector.tensor_mul(out=sq, in0=xb, in1=xb)
                    else:
                        nc.gpsimd.tensor_copy(out=xb, in_=xs)
                        nc.gpsimd.tensor_mul(out=sq, in0=xb, in1=xb)
                    nc.tensor.matmul(ssum, ones_col, xb,
                                     start=(eo == 0), stop=(eo == EO - 1))
                    nc.tensor.matmul(ssq, ones_col, sq,
                                     start=(eo == 0), stop=(eo == EO - 1))
                # fused row math: rstd = 1/sqrt(ssq/E - (ssum/E)^2 + eps),
                # crow = -(ssum/E) * rstd; collected in rc2 rows [rstd; crow]
                msq = rowpool.tile([1, CH], F32, name="msq", tag="msq", bufs=2)
                nc.scalar.activation(out=msq, in_=ssum, func=AF.Square,
                                     scale=1.0 / E)
                nc.vector.scalar_tensor_tensor(
                    out=msq, in0=ssq, scalar=1.0 / E, in1=msq,
                    op0=ALU.mult, op1=ALU.subtract)
                nc.scalar.activation(out=msq, in_=msq, func=AF.Sqrt,
                                     scale=(1.0 / 1024.0 if q8 else 1.0),
                                     bias=(eps_cq if q8 else eps_c))
                rstd_r = rowpool.tile([1, CH], BF16, name="rstd_r",
                                      tag="rstd_r", bufs=1)
                with nc.allow_low_precision(reason="rstd/crow rows in bf16"):
                    nc.vector.reciprocal(out=rstd_r, in_=msq)
                crow_r = rowpool.tile([1, CH], BF16, name="crow_r",
                                      tag="crow_r", bufs=1)
                nc.vector.scalar_tensor_tensor(
                    out=crow_r, in0=ssum, scalar=-1.0 / E, in1=rstd_r,
                    op0=ALU.mult, op1=ALU.mult)
                a_ps = psum([P, CH], "a_ps", "ops", 2)
                nc.tensor.matmul(a_ps, ones_row_bf, rstd_r,
                                 start=True, stop=True)
                c_ps = psum([P, CH], "c_ps", "ops", 2)
                nc.tensor.matmul(c_ps, ones_row_bf, crow_r,
                                 start=True, stop=True)
                a_sb = lnpool.tile([P, CH], BF16, name="a_sb", tag="a_sb",
                                   bufs=1)
                nc.scalar.copy(out=a_sb, in_=a_ps)
                c_sb = lnpool.tile([P, CH], BF16, name="c_sb", tag="c_sb",
                                   bufs=1)
                nc.scalar.copy(out=c_sb, in_=c_ps)
                acs.append((a_ps, c_ps, a_sb, c_sb))
            for c in range(NCH):
                csl = bass.ts(c, CH)
                a_ps, c_ps, a_sb, c_sb = acs[c]
                for eo in range(EO):
                    t1 = lnpool.tile([P, CH], F32, name="lnt1", tag="lnt1",
                                     bufs=2)
                    on_dve = (eo // 2) % 2 == 0
                    eng = nc.vector if on_dve else nc.gpsimd
                    am, cm = (a_ps, c_ps) if on_dve else (a_sb, c_sb)
                    eng.tensor_mul(out=t1, in0=x_sb[:, eo, csl], in1=am)
                    if g is None:
                        with nc.allow_low_precision(reason="h out fp8/bf16"):
                            eng.tensor_add(out=out[:, eo, csl], in0=t1,
                                           in1=cm)
                    else:
                        eng.tensor_add(out=t1, in0=t1, in1=cm)
                        with nc.allow_low_precision(reason="h out fp8/bf16"):
                            eng.tensor_scalar(
                                out=out[:, eo, csl], in0=t1,
                                scalar1=g[:, eo:eo + 1], scalar2=b[:, eo:eo + 1],
                                op0=ALU.mult, op1=ALU.add)
            ctx2.__exit__(None, None, None)
            return out

        def load_strip(w2d, r0, rn, c0, cn, nm, pool, tag=None, dtype=BF16,
                       eng=None):
            t = pool.tile([P, rn // P, cn], dtype, name=nm, tag=tag or nm)
            src = w2d[r0:r0 + rn, c0:c0 + cn].rearrange(
                "(ko p) n -> p ko n", p=P)
            # attention-projection strips ride the SP HW DGE queue, MLP
            # strips the ACT queue, so a layer's first qkv strip is not
            # queued behind the previous layer's MLP weight stream
            if eng is None:
                eng = nc.sync
            with tc.high_priority():
                eng.dma_start(out=t, in_=src)
            return t

        def proj_fm(rhs_sb, w_l, evict, dr=False):
            strips = [load_strip(w_l, 0, E, nh * WS, WS, "wproj", wpool,
                                 dtype=FP8 if dr else BF16)
                      for nh in range(E // WS)]
            for c in range(NCH):
                for nh in range(E // WS):
                    wt = strips[nh]
                    for ni in range(WS // P):
                        no = nh * (WS // P) + ni
                        pst = psum([P, CH], "pss", "psb", 3)
                        if dr:
                            for k2 in range(EO // 2):
                                nc.tensor.matmul(
                                    pst,
                                    wt[:, 2 * k2:2 * k2 + 2,
                                       ni * P:(ni + 1) * P],
                                    rhs_sb[:, 2 * k2:2 * k2 + 2,
                                           bass.ts(c, CH)],
                                    start=(k2 == 0),
                                    stop=(k2 == EO // 2 - 1),
                                    perf_mode=mybir.MatmulPerfMode.DoubleRow)
                        else:
                            for ko in range(EO):
                                nc.tensor.matmul(
                                    pst, wt[:, ko, ni * P:(ni + 1) * P],
                                    rhs_sb[:, ko, bass.ts(c, CH)],
                                    start=(ko == 0), stop=(ko == EO - 1))
                        evict(pst, no, c)

        for rep in range(int(cfg.get("repeat", 1))):
          for c in range(NCH):
              nc.sync.dma_start(
                  out=x_sb[:, :, bass.ts(c, CH)],
                  in_=x_d[:, bass.ts(c, CH)].rearrange("(o p) t -> p o t",
                                                       p=P))
          for l in range(L):
              h_sb = layernorm(params.get(("ln1_g", l)), params.get(("ln1_b", l)),
                               q8=True)

              # v: token-major [P, to, H, HD+1]; trailing ones column makes
              # the A@V matmul emit the softmax denominator as out row HD
              v_sb = vhpool.tile([P, TOK // P, H, HD + 1], FP8, name="v_sb",
                                 tag="vh")
              nc.vector.memset(v_sb[:, :, :, HD:HD + 1], 32.0)
              bvb = None
              if "bv" in extra:
                  bvrow = rowpool.tile([1, E], F32, name="bvrow", tag="bvrow")
                  nc.sync.dma_start(out=bvrow,
                                    in_=extra["bv"][l].rearrange("e -> 1 e"))
                  bvb = rowpool.tile([P, E], F32, name="bvb", tag="bvb")
                  for j in range(E // CH):
                      bp = psum([P, CH], "bvps", "psb", 3)
                      nc.tensor.matmul(bp, ones_row, bvrow[:, bass.ts(j, CH)],
                                       start=True, stop=True)
                      nc.scalar.copy(out=bvb[:, bass.ts(j, CH)], in_=bp)
              wvs = [load_strip(wv_d[l], 0, E, j * WS, WS, "wproj", wpool,
                                dtype=FP8)
                     for j in range(E // WS)]
              for to in range(TOK // P):
                  pss = [psum([P, WS], "pss", "psb", 3) for _ in range(E // WS)]
                  for k2 in range(EO // 2):
                      for j in range(E // WS):
                          nc.tensor.matmul(
                              pss[j],
                              h_sb[:, 2 * k2:2 * k2 + 2, to * P:(to + 1) * P],
                              wvs[j][:, 2 * k2:2 * k2 + 2, :],
                              start=(k2 == 0), stop=(k2 == EO // 2 - 1),
                              perf_mode=mybir.MatmulPerfMode.DoubleRow)
                  hpw = WS // HD
                  for j in range(E // WS):
                      dst = v_sb[:, to, j * hpw:(j + 1) * hpw, :HD]
                      with nc.allow_low_precision(reason="v8 fp8"):
                          if bvb is None:
                              nc.vector.tensor_scalar_mul(
                                  dst, pss[j], scl_sb[:, 6 * l + 2:6 * l + 3])
                          else:
                              nc.vector.tensor_add(out=dst, in0=pss[j],
                                                   in1=bvb[:, bass.ts(j, WS)])

              # q/k projections per strip, then that strip's heads' attention
              ao_sb = h_sb if cfg.get("noattn") else hpool.tile(
                  [P, EO, TOK], BF16, name="hs", tag="hs")
              ao8_sb = hpool.tile([P, EO, TOK], FP8, name="ao8", tag="ao8",
                                  bufs=1)
              pending_norm = []

              def flush_norms():
                  for eo_hp, rr2p in pending_norm:
                      for b in range(BL):
                          rb_ps = psum([P, T], "rbps", "psb", 3)
                          nc.tensor.matmul(rb_ps, onesA, rr2p[(0, b)],
                                           start=True, stop=False)
                          nc.tensor.matmul(rb_ps, onesB, rr2p[(1, b)],
                                           start=False, stop=True)
                          sl = ao_sb[:, eo_hp, b * T:(b + 1) * T]
                          with nc.allow_low_precision(reason="ao8 fp8"):
                              nc.vector.tensor_mul(
                                  out=ao8_sb[:, eo_hp, b * T:(b + 1) * T],
                                  in0=sl, in1=rb_ps)
                  pending_norm.clear()

              for nh in range(E // WQK):
                q_sb = qkpool.tile([P, WQK // P, TOK], BF16, name="qs",
                                   tag="qk", bufs=4)
                k_sb = qkpool.tile([P, WQK // P, TOK], BF16, name="ks",
                                   tag="qk", bufs=4)
                wqs = load_strip(wq_d[l], 0, E, nh * WQK, WQK, "wproj", wpool,
                                 dtype=FP8)
                wks = load_strip(wk_d[l], 0, E, nh * WQK, WQK, "wproj", wpool,
                                 dtype=FP8)
                for wt, dst, si, bt_p in (
                        (wqs, q_sb, 0, params.get(("bq", l))),
                        (wks, k_sb, 1, params.get(("bk", l)))):
                    for ni in range(WQK // P):
                        no = nh * (WQK // P) + ni
                        pss = [psum([P, CH], "pss", "psb", 3)
                               for _ in range(NCH)]
                        for k2 in range(EO // 2):
                            for c in range(NCH):
                                nc.tensor.matmul(
                                    pss[c],
                                    wt[:, 2 * k2:2 * k2 + 2,
                                       ni * P:(ni + 1) * P],
                                    h_sb[:, 2 * k2:2 * k2 + 2, bass.ts(c, CH)],
                                    start=(k2 == 0), stop=(k2 == EO // 2 - 1),
                                    perf_mode=mybir.MatmulPerfMode.DoubleRow)
                        for c in range(NCH):
                            if bt_p is None:
                                nc.vector.tensor_scalar_mul(
                                    dst[:, ni, bass.ts(c, CH)], pss[c],
                                    scl_sb[:, 6 * l + si:6 * l + si + 1])
                            else:
                                nc.scalar.activation(
                                    out=dst[:, ni, bass.ts(c, CH)], in_=pss[c],
                                    func=AF.Identity, bias=bt_p[:, no:no + 1],
                                    scale=scl_sb[:, 6 * l + si:6 * l + si + 1])
                hs0 = nh * WQK // HD
                hs1 = (nh + 1) * WQK // HD
                rr2s = {}
                if nh > 0:
                    flush_norms()
                for h in range(hs0, hs0 if cfg.get("noattn") else hs1):
                  po = (h % HPT) * HD
                  eo_h = h // HPT
                  ni_h = eo_h - nh * (WQK // P)
                  # bias tiles bt[p, tk, tq] = exp(bias[tq, tk*P+p]) (transposed)
                  bt = bpool.tile([P, TQ, T], BF16, name="btile", tag="btile")
                  with tc.high_priority():
                      nc.gpsimd.dma_start(
                          out=bt, in_=bm_d[l, h].rearrange("(tk p) t -> p tk t",
                                                           p=P))
                  if h % 2 == 0:
                      rr2s = {(par, b): rowpool.tile([1, T], BF16, name="rr2",
                                                     tag="rr2", bufs=6)
                              for par in range(2) for b in range(BL)}
                      pending_norm.append((eo_h, rr2s))
                  pts = ptpool.tile([P, TQ, BL, T], FP8, name="pts",
                                    tag="pts")
                  for tk in range(TQ):
                      for b in range(BL):
                          sps = psum([P, T], "sps", "sps", 3)
                          nc.tensor.matmul(
                              sps,
                              k_sb[po:po + HD, ni_h,
                                   b * T + tk * P: b * T + (tk + 1) * P],
                              q_sb[po:po + HD, ni_h, b * T: (b + 1) * T],
                              start=True, stop=True)
                          # pts = 8*exp(s) * exp(bias): exp on ACT (bf16),
                          # bias multiply on Pool (PE stays out of it)
                          ptm = ptpool.tile([P, T], BF16, name="ptm",
                                            tag="ptm", bufs=3)
                          nc.scalar.activation(out=ptm, in_=sps,
                                               func=AF.Exp, bias=ln16_c)
                          with nc.allow_low_precision(reason="pts fp8"):
                              nc.gpsimd.tensor_mul(out=pts[:, tk, b],
                                                   in0=ptm, in1=bt[:, tk, :])
                  for b in range(BL):
                      ops = psum([HD + 1, T], "ops", "ops", 2)
                      for t2 in range(TQ // 2):
                          nc.tensor.matmul(
                              ops,
                              v_sb[:, b * TQ + 2 * t2:b * TQ + 2 * t2 + 2,
                                   h, :],
                              pts[:, 2 * t2:2 * t2 + 2, b, :],
                              start=(t2 == 0), stop=(t2 == TQ // 2 - 1),
                              perf_mode=mybir.MatmulPerfMode.DoubleRow)
                      nc.vector.tensor_copy(
                          out=ao_sb[po:po + HD, eo_h, b * T:(b + 1) * T],
                          in_=ops[:HD, :])
                      with nc.allow_low_precision(reason="recip rows bf16"):
                          nc.vector.reciprocal(
                              out=rr2s[(h % 2, b)], in_=ops[HD:HD + 1, :])


              flush_norms()

              # out projection + residual (in place)
              bo_t = params.get(("bo", l))

              def o_evict(pst, no, c):
                  csl = bass.ts(c, CH)
                  nc.vector.scalar_tensor_tensor(
                      out=x_sb[:, no, csl], in0=pst,
                      scalar=scl_sb[:, 6 * l + 3:6 * l + 4],
                      in1=x_sb[:, no, csl], op0=ALU.mult, op1=ALU.add)
              proj_fm(ao8_sb, wo_d[l], o_evict, dr=True)

              h2_sb = layernorm(params.get(("ln2_g", l)),
                                params.get(("ln2_b", l)))

              # MLP per token chunk (bf16: fp8 anywhere in the MLP costs
              # ~2.1% output error per quantization point -- over budget)
              b1_t = params.get(("b1", l))
              b2_t = params.get(("b2", l))
              for c in range(NCH):
                  csl = bass.ts(c, CH)
                  hid_sb = vhpool.tile([P, FO, CH], BF16, name="hid",
                                       tag="vh")
                  fblk = min(4, FO)
                  for fb in range(FO // fblk):
                      w1c = load_strip(w1_d[l], 0, E, fb * fblk * P,
                                       fblk * P, "w1c", wpool, tag="wproj",
                                       eng=nc.scalar)
                      for ni in range(fblk):
                          fo = fb * fblk + ni
                          pst = psum([P, CH], "pss", "psb", 3)
                          for ko in range(EO):
                              nc.tensor.matmul(
                                  pst, w1c[:, ko, ni * P:(ni + 1) * P],
                                  h2_sb[:, ko, csl],
                                  start=(ko == 0), stop=(ko == EO - 1))
                          nc.scalar.activation(
                              out=hid_sb[:, fo, :], in_=pst,
                              func=(AF.Tanh if cfg.get("act") == "tanh"
                                    else AF.Gelu),
                              bias=(0.0 if b1_t is None
                                    else b1_t[:, fo:fo + 1]))
                  nblk = min(2, EO)
                  kh_n = 2 if FO % 2 == 0 else 1
                  for nb in range(EO // nblk):
                    w2cs = [load_strip(w2_d[l], kh * (FF // kh_n), FF // kh_n,
                                       nb * nblk * P, nblk * P, "w2c", w2pool,
                                       eng=nc.scalar)
                            for kh in range(kh_n)]
                    for no_i in range(nblk):
                      no = nb * nblk + no_i
                      pst = psum([P, CH], "pss", "psb", 3)
                      for kh in range(kh_n):
                          for ko in range(FO // kh_n):
                              nc.tensor.matmul(
                                  pst, w2cs[kh][:, ko, no_i * P:(no_i + 1) * P],
                                  hid_sb[:, kh * (FO // kh_n) + ko, :],
                                  start=(kh == 0 and ko == 0),
                                  stop=(kh == kh_n - 1 and
                                        ko == FO // kh_n - 1))
                      if b2_t is None:
                          nc.vector.tensor_add(out=x_sb[:, no, csl], in0=pst,
                                               in1=x_sb[:, no, csl])
                      else:
                          nc.vector.scalar_tensor_tensor(
                              out=x_sb[:, no, csl], in0=pst,
                              scalar=b2_t[:, no:no + 1], in1=x_sb[:, no, csl],
                              op0=ALU.add, op1=ALU.add)

        nc.sync.dma_start(out=out_d.rearrange("(o p) t -> p o t", p=P),
                          in_=x_sb)

    nc.finalize()
    return nc


def host_prep(inputs, cfg):
    BL, T, E, H, FF, L = cfg["BL"], cfg["T"], cfg["E"], cfg["H"], cfg["FF"], cfg["L"]
    HD = E // H
    bf = ml_dtypes.bfloat16
    f32 = np.float32
    inp = {k: np.asarray(v, dtype=np.float32) for k, v in inputs.items()}

    f8 = ml_dtypes.float8_e4m3fn
    shared = {
        "w1": inp["w1"].astype(bf),
        "w2": inp["w2"].astype(bf),
    }
    scl = np.zeros((L, 6), np.float32)
    for nm, si, arr in (("wq", 0, inp["wq"] * (HD ** -0.5)), ("wk", 1, inp["wk"]),
                        ("wv", 2, inp["wv"]), ("wo", 3, inp["wo"])):
        q = np.empty_like(arr, dtype=f8)
        for l in range(L):
            sw = 224.0 / max(np.abs(arr[l]).max(), 1e-30)
            q[l] = (arr[l] * sw).astype(f8)
            # eviction descale: q,k,o fold 1/32 (h8 = 32*h); v folds 32/32
            scl[l, si] = (1.0 / (32.0 * sw)) if si != 2 else (1.0 / sw)
        shared[nm] = q
    shared["scl"] = np.broadcast_to(
        scl.reshape(1, L * 6), (128, L * 6)).copy()
    coords = np.arange(T)
    rel = (coords[:, None] - coords[None, :] + (T - 1)).astype(np.float64)
    bias_all = np.empty((L, H, T, T), dtype=bf)
    for l in range(L):
        off = np.tanh(np.float64(inp["offset"][l, 0])) * MAX_OFFSET
        adj = np.clip(rel + off, 0.0, 2.0 * T - 2.0)
        lo = np.floor(adj).astype(np.int64)
        hi = np.ceil(adj).astype(np.int64)
        w = (adj - lo)[..., None].astype(f32)
        tab = inp["bias_table"][l]
        bm = tab[lo] * (1.0 - w) + tab[hi] * w
        # transposed layout: biasmat[l, h, key_pos, query_pos] = exp(bias)
        # (bias folded into the post-exp multiply on Pool)
        bias_all[l] = np.exp(bm.transpose(2, 1, 0)).astype(bf)
    shared["biasmat"] = bias_all

    flags = set()
    for nm, arr in [("bq", (inp["bq"] * (HD ** -0.5)).astype(f32)),
                    ("bk", inp["bk"]), ("bv", inp["bv"]), ("bo", inp["bo"]),
                    ("b1", inp["b1"]), ("b2", inp["b2"])]:
        if np.any(arr):
            flags.add(nm)
            shared[nm] = np.ascontiguousarray(arr, dtype=f32)
    for pre in ("ln1", "ln2"):
        if np.any(inp[f"{pre}_g"] != 1.0) or np.any(inp[f"{pre}_b"]):
            flags.add(pre)
            shared[f"{pre}_g"] = inp[f"{pre}_g"].astype(f32)
            shared[f"{pre}_b"] = inp[f"{pre}_b"].astype(f32)

    per_core_x = []
    for c in range(N_CORES):
        xs = inp["x"][c * BL:(c + 1) * BL]
        per_core_x.append(np.ascontiguousarray(
            xs.transpose(2, 0, 1).reshape(E, BL * T)))
    return shared, per_core_x, frozenset(flags)


_CACHE = {}


def kernel(**inputs) -> np.ndarray:
    cfg = FULL
    BL, T, E = cfg["BL"], cfg["T"], cfg["E"]
    shared, per_core_x, flags = host_prep(inputs, cfg)
    key = ("full", flags)
    if key not in _CACHE:
        _CACHE[key] = build_nc(cfg, flags)
    nc = _CACHE[key]
    in_maps = [{"x_fm": per_core_x[c], **shared} for c in range(N_CORES)]
    res = run_bass_kernel_spmd(nc, in_maps, core_ids=list(range(N_CORES)))
    out = np.empty((N_CORES * BL, T, E), np.float32)
    for c in range(N_CORES):
        ofm = res.results[c]["out_fm"]
        out[c * BL:(c + 1) * BL] = ofm.reshape(E, BL, T).transpose(1, 2, 0)
    return out



# revision 30
# speedup vs baseline: 1.1738x; 1.1738x over previous
"""Trainium2 Bass kernel: 4-layer pose-temporal transformer encoder.

kernel(**inputs) takes FULL unsharded fp32 inputs, returns FULL (16,512,1024)
fp32 output.  Data-parallel over batch across 8 NeuronCores (2 batch elements
per core, no collectives).

Per-core: feature-major fp32 residual x[E(part), tokens].  QKV/out
projections and A@V run in fp8(e4m3) with DoubleRow perf mode (per-tensor
host-side weight scaling, descale factors streamed as a DRAM param so the
NEFF is value-independent); MLP and attention scores stay bf16 with fp32
PSUM accumulation (fp8 anywhere in the MLP costs ~2.1% output error per
quantization point -- over the 2e-2 budget).  The relative-position bias
is precomputed on the host as exp(bias) in a (L,H,T,T) bf16 table and
folded in AFTER the exp: ACT emits 8*exp(s) to bf16, Pool multiplies by
exp(bias) into fp8 (keeps the PE out of the bias entirely); the
ones-column of V yields the softmax denominators, applied as a
paired-head broadcast matmul + multiply.  LayerNorm statistics use
matmul partition-reductions with copies/squares spread across
DVE/ACT/Pool and a fused row chain; stats for all chunks are emitted
before applies so engine FIFOs never block chunk c+1 behind chunk c.
Weight/bias streams are double-buffered and spread over both HW DGE
queues (SP + ACT) plus the Pool SWDGE.
"""

import numpy as np
import ml_dtypes
from contextlib import ExitStack

import concourse.bass as bass
import concourse.tile as tile
from concourse import bacc, mybir
from concourse.bass_utils import run_bass_kernel_spmd

F32 = mybir.dt.float32
F32R = mybir.dt.float32r
FP8 = mybir.dt.float8e4
LN16 = 2.0794415416798357  # ln(8): pts8 = 8*exp(s), max ~205 < 448
BF16 = mybir.dt.bfloat16
AF = mybir.ActivationFunctionType
ALU = mybir.AluOpType
P = 128

FULL = dict(BL=2, T=512, E=1024, H=16, FF=4096, L=4)
N_CORES = 8
EPS = 1e-5
MAX_OFFSET = 0.5


def build_nc(cfg, flags=frozenset()):
    BL, T, E, H, FF, L = cfg["BL"], cfg["T"], cfg["E"], cfg["H"], cfg["FF"], cfg["L"]
    HD = E // H
    EO = E // P
    FO = FF // P
    TOK = BL * T
    CH = min(512, T)
    NCH = TOK // CH
    TQ = T // P
    HPT = max(1, P // HD)
    WS = min(512, E)
    WQK = min(256, E)       # q/k strip width (SBUF pressure)

    nc = bacc.Bacc(None, target_bir_lowering=False,
                   debug=bool(cfg.get("debug", False)))

    x_d = nc.declare_dram_parameter("x_fm", [E, TOK], F32, False)
    wq_d = nc.declare_dram_parameter("wq", [L, E, E], FP8, False)
    wk_d = nc.declare_dram_parameter("wk", [L, E, E], FP8, False)
    wv_d = nc.declare_dram_parameter("wv", [L, E, E], FP8, False)
    wo_d = nc.declare_dram_parameter("wo", [L, E, E], FP8, False)
    scl_d = nc.declare_dram_parameter("scl", [P, L * 6], F32, False)
    w1_d = nc.declare_dram_parameter("w1", [L, E, FF], BF16, False)
    w2_d = nc.declare_dram_parameter("w2", [L, FF, E], BF16, False)
    bm_d = nc.declare_dram_parameter("biasmat", [L, H, T, T], BF16, False)
    extra = {}
    for nm, shp in [("bq", [L, E]), ("bk", [L, E]), ("bv", [L, E]),
                    ("bo", [L, E]), ("b1", [L, FF]), ("b2", [L, E]),
                    ("ln1_g", [L, E]), ("ln1_b", [L, E]),
                    ("ln2_g", [L, E]), ("ln2_b", [L, E])]:
        key = nm.split("_")[0] if nm.startswith("ln") else nm
        if key in flags:
            extra[nm] = nc.declare_dram_parameter(nm, shp, F32, False)
    out_d = nc.declare_dram_parameter("out_fm", [E, TOK], F32, True)

    with tile.TileContext(nc) as tc, ExitStack() as ctx:
        const = ctx.enter_context(tc.tile_pool(name="const", bufs=1))
        resid = ctx.enter_context(tc.tile_pool(name="resid", bufs=1))
        hpool = ctx.enter_context(tc.tile_pool(name="hpool", bufs=2))
        qkpool = ctx.enter_context(tc.tile_pool(name="qkpool", bufs=2))
        vhpool = ctx.enter_context(tc.tile_pool(name="vhpool", bufs=1))
        wpool = ctx.enter_context(tc.tile_pool(name="wpool", bufs=2))
        w2pool = ctx.enter_context(tc.tile_pool(name="w2pool", bufs=3))
        lnpool = ctx.enter_context(tc.tile_pool(name="lnpool", bufs=2))
        rowpool = ctx.enter_context(tc.tile_pool(name="rowpool", bufs=1))
        ptpool = ctx.enter_context(tc.tile_pool(name="ptpool", bufs=2))
        bpool = ctx.enter_context(tc.tile_pool(name="bpool", bufs=2))
        ps = ctx.enter_context(tc.tile_pool(name="ps", bufs=1, space="PSUM"))

        def psum(shape, name, tag, bufs):
            return ps.tile(shape, F32, name=name, tag=tag, bufs=bufs)

        ones_col = const.tile([P, 1], BF16)
        nc.vector.memset(ones_col, 1.0)
        ones_col_f = const.tile([P, 1], F32)
        nc.vector.memset(ones_col_f, 1.0)
        ones_row = const.tile([1, P], F32)
        nc.vector.memset(ones_row, 1.0)
        ones_row_bf = const.tile([1, P], BF16)
        nc.vector.memset(ones_row_bf, 1.0)
        onesA = const.tile([1, P], BF16)    # ones on cols 0..HD-1
        nc.vector.memset(onesA, 0.0)
        nc.vector.memset(onesA[0:1, 0:HD], 32.0)
        onesB = const.tile([1, P], BF16)    # ones on cols HD..127
        nc.vector.memset(onesB, 0.0)
        nc.vector.memset(onesB[0:1, HD:2 * HD], 32.0)
        eps_c = const.tile([1, 1], F32)
        nc.vector.memset(eps_c, EPS)
        eps_cq = const.tile([1, 1], F32)
        nc.vector.memset(eps_cq, EPS / 1024.0)
        ln16_c = const.tile([P, 1], F32)
        nc.vector.memset(ln16_c, LN16)
        scl_sb = const.tile([P, L * 6], F32, name="scl_sb", tag="scl_sb")
        nc.sync.dma_start(out=scl_sb, in_=scl_d[:, :])

        def load_param_cols(dram_row, n_tiles, nm):
            t = const.tile([P, n_tiles], F32, name=nm, tag=nm)
            nc.sync.dma_start(out=t, in_=dram_row.rearrange("(o p) -> p o", p=P))
            return t

        params = {}
        for l in range(L):
            for nm in ("bq", "bk", "bo", "b1", "b2"):
                if nm in extra:
                    n_t = FO if nm == "b1" else EO
                    params[(nm, l)] = load_param_cols(extra[nm][l], n_t, f"{nm}{l}")
            for nm in ("ln1_g", "ln1_b", "ln2_g", "ln2_b"):
                if nm in extra:
                    params[(nm, l)] = load_param_cols(extra[nm][l], EO, f"{nm}{l}")

        x_sb = resid.tile([P, EO, TOK], F32)

        def layernorm(g, b, q8=False):
            """LN of x_sb (feature-major); stats for ALL chunks emitted
            first (so chunk c+1 stats aren't queued behind chunk c apply),
            then the applies.  q8: emit fp8 h scaled by 32 (folded into
            rstd via sqrt((va+eps)/1024))."""
            out = hpool.tile([P, EO, TOK], FP8 if q8 else BF16,
                             name="hs", tag="hs")
            ctx2 = tc.high_priority()
            ctx2.__enter__()
            acs = []
            for c in range(NCH):
                csl = bass.ts(c, CH)
                ssum = psum([1, CH], "ssum", "sps", 2)
                ssq = psum([1, CH], "ssq", "sps", 2)
                # copies+squares split over DVE / ACT / Pool; stats MMs
                # interleave per eo
                for eo in range(EO):
                    xb = lnpool.tile([P, CH], BF16, name="xbc", tag="xbc",
                                     bufs=2)
                    sq = lnpool.tile([P, CH], BF16, name="sqc", tag="sqc",
                                     bufs=2)
                    xs = x_sb[:, eo, csl]
                    if eo < 4:
                        nc.vector.tensor_copy(out=xb, in_=xs)
                        nc.vector.tensor_mul(out=sq, in0=xb, in1=xb)
                    elif eo < 6:
                        nc.scalar.copy(out=xb, in_=xs)
                        nc.scalar.activation(out=sq, in_=xs, func=AF.Square)
                    else:
                        nc.gpsimd.tensor_copy(out=xb, in_=xs)
                        nc.gpsimd.tensor_mul(out=sq, in0=xb, in1=xb)
                    nc.tensor.matmul(ssum, ones_col, xb,
                                     start=(eo == 0), stop=(eo == EO - 1))
                    nc.tensor.matmul(ssq, ones_col, sq,
                                     start=(eo == 0), stop=(eo == EO - 1))
                # fused row math: rstd = 1/sqrt(ssq/E - (ssum/E)^2 + eps),
                # crow = -(ssum/E) * rstd; collected in rc2 rows [rstd; crow]
                msq = rowpool.tile([1, CH], F32, name="msq", tag="msq", bufs=2)
                nc.scalar.activation(out=msq, in_=ssum, func=AF.Square,
                                     scale=1.0 / E)
                nc.vector.scalar_tensor_tensor(
                    out=msq, in0=ssq, scalar=1.0 / E, in1=msq,
                    op0=ALU.mult, op1=ALU.subtract)
                nc.scalar.activation(out=msq, in_=msq, func=AF.Sqrt,
                                     scale=(1.0 / 1024.0 if q8 else 1.0),
                                     bias=(eps_cq if q8 else eps_c))
                rstd_r = rowpool.tile([1, CH], BF16, name="rstd_r",
                                      tag="rstd_r", bufs=1)
                with nc.allow_low_precision(reason="rstd/crow rows in bf16"):
                    nc.vector.reciprocal(out=rstd_r, in_=msq)
                crow_r = rowpool.tile([1, CH], BF16, name="crow_r",
                                      tag="crow_r", bufs=1)
                nc.vector.scalar_tensor_tensor(
                    out=crow_r, in0=ssum, scalar=-1.0 / E, in1=rstd_r,
                    op0=ALU.mult, op1=ALU.mult)
                a_ps = psum([P, CH], "a_ps", "ops", 2)
                nc.tensor.matmul(a_ps, ones_row_bf, rstd_r,
                                 start=True, stop=True)
                c_ps = psum([P, CH], "c_ps", "ops", 2)
                nc.tensor.matmul(c_ps, ones_row_bf, crow_r,
                                 start=True, stop=True)
                a_sb = lnpool.tile([P, CH], BF16, name="a_sb", tag="a_sb",
                                   bufs=1)
                nc.scalar.copy(out=a_sb, in_=a_ps)
                c_sb = lnpool.tile([P, CH], BF16, name="c_sb", tag="c_sb",
                                   bufs=1)
                nc.scalar.copy(out=c_sb, in_=c_ps)
                acs.append((a_ps, c_ps, a_sb, c_sb))
            for c in range(NCH):
                csl = bass.ts(c, CH)
                a_ps, c_ps, a_sb, c_sb = acs[c]
                for eo in range(EO):
                    t1 = lnpool.tile([P, CH], F32, name="lnt1", tag="lnt1",
                                     bufs=2)
                    on_dve = (eo // 2) % 2 == 0
                    eng = nc.vector if on_dve else nc.gpsimd
                    am, cm = (a_ps, c_ps) if on_dve else (a_sb, c_sb)
                    eng.tensor_mul(out=t1, in0=x_sb[:, eo, csl], in1=am)
                    if g is None:
                        with nc.allow_low_precision(reason="h out fp8/bf16"):
                            eng.tensor_add(out=out[:, eo, csl], in0=t1,
                                           in1=cm)
                    else:
                        eng.tensor_add(out=t1, in0=t1, in1=cm)
                        with nc.allow_low_precision(reason="h out fp8/bf16"):
                            eng.tensor_scalar(
                                out=out[:, eo, csl], in0=t1,
                                scalar1=g[:, eo:eo + 1], scalar2=b[:, eo:eo + 1],
                                op0=ALU.mult, op1=ALU.add)
            ctx2.__exit__(None, None, None)
            return out

        _dmaq = [nc.sync, nc.scalar]
        _dman = [0]

        def load_strip(w2d, r0, rn, c0, cn, nm, pool, tag=None, dtype=BF16):
            t = pool.tile([P, rn // P, cn], dtype, name=nm, tag=tag or nm)
            src = w2d[r0:r0 + rn, c0:c0 + cn].rearrange(
                "(ko p) n -> p ko n", p=P)
            eng = _dmaq[_dman[0] % len(_dmaq)]
            _dman[0] += 1
            with tc.high_priority():
                eng.dma_start(out=t, in_=src)
            return t

        def proj_fm(rhs_sb, w_l, evict, dr=False):
            strips = [load_strip(w_l, 0, E, nh * WS, WS, "wproj", wpool,
                                 dtype=FP8 if dr else BF16)
                      for nh in range(E // WS)]
            for c in range(NCH):
                for nh in range(E // WS):
                    wt = strips[nh]
                    for ni in range(WS // P):
                        no = nh * (WS // P) + ni
                        pst = psum([P, CH], "pss", "psb", 4)
                        if dr:
                            for k2 in range(EO // 2):
                                nc.tensor.matmul(
                                    pst,
                                    wt[:, 2 * k2:2 * k2 + 2,
                                       ni * P:(ni + 1) * P],
                                    rhs_sb[:, 2 * k2:2 * k2 + 2,
                                           bass.ts(c, CH)],
                                    start=(k2 == 0),
                                    stop=(k2 == EO // 2 - 1),
                                    perf_mode=mybir.MatmulPerfMode.DoubleRow)
                        else:
                            for ko in range(EO):
                                nc.tensor.matmul(
                                    pst, wt[:, ko, ni * P:(ni + 1) * P],
                                    rhs_sb[:, ko, bass.ts(c, CH)],
                                    start=(ko == 0), stop=(ko == EO - 1))
                        evict(pst, no, c)

        for rep in range(int(cfg.get("repeat", 1))):
          for c in range(NCH):
              nc.sync.dma_start(
                  out=x_sb[:, :, bass.ts(c, CH)],
                  in_=x_d[:, bass.ts(c, CH)].rearrange("(o p) t -> p o t",
                                                       p=P))
          for l in range(L):
              h_sb = layernorm(params.get(("ln1_g", l)), params.get(("ln1_b", l)),
                               q8=True)

              # v: token-major [P, to, H, HD+1]; trailing ones column makes
              # the A@V matmul emit the softmax denominator as out row HD
              v_sb = vhpool.tile([P, TOK // P, H, HD + 1], FP8, name="v_sb",
                                 tag="vh")
              nc.vector.memset(v_sb[:, :, :, HD:HD + 1], 32.0)
              bvb = None
              if "bv" in extra:
                  bvrow = rowpool.tile([1, E], F32, name="bvrow", tag="bvrow")
                  nc.sync.dma_start(out=bvrow,
                                    in_=extra["bv"][l].rearrange("e -> 1 e"))
                  bvb = rowpool.tile([P, E], F32, name="bvb", tag="bvb")
                  for j in range(E // CH):
                      bp = psum([P, CH], "bvps", "psb", 4)
                      nc.tensor.matmul(bp, ones_row, bvrow[:, bass.ts(j, CH)],
                                       start=True, stop=True)
                      nc.scalar.copy(out=bvb[:, bass.ts(j, CH)], in_=bp)
              wvs = [load_strip(wv_d[l], 0, E, j * WS, WS, "wproj", wpool,
                                dtype=FP8)
                     for j in range(E // WS)]
              for to in range(TOK // P):
                  pss = [psum([P, WS], "pss", "psb", 4) for _ in range(E // WS)]
                  for k2 in range(EO // 2):
                      for j in range(E // WS):
                          nc.tensor.matmul(
                              pss[j],
                              h_sb[:, 2 * k2:2 * k2 + 2, to * P:(to + 1) * P],
                              wvs[j][:, 2 * k2:2 * k2 + 2, :],
                              start=(k2 == 0), stop=(k2 == EO // 2 - 1),
                              perf_mode=mybir.MatmulPerfMode.DoubleRow)
                  hpw = WS // HD
                  for j in range(E // WS):
                      dst = v_sb[:, to, j * hpw:(j + 1) * hpw, :HD]
                      with nc.allow_low_precision(reason="v8 fp8"):
                          if bvb is None:
                              nc.vector.tensor_scalar_mul(
                                  dst, pss[j], scl_sb[:, 6 * l + 2:6 * l + 3])
                          else:
                              nc.vector.tensor_add(out=dst, in0=pss[j],
                                                   in1=bvb[:, bass.ts(j, WS)])

              # q/k projections per strip, then that strip's heads' attention
              ao_sb = h_sb if cfg.get("noattn") else hpool.tile(
                  [P, EO, TOK], BF16, name="hs", tag="hs")
              ao8_sb = hpool.tile([P, EO, TOK], FP8, name="ao8", tag="ao8",
                                  bufs=1)
              pending_norm = []

              def flush_norms():
                  for eo_hp, rr2p in pending_norm:
                      for b in range(BL):
                          rb_ps = psum([P, T], "rbps", "psb", 4)
                          nc.tensor.matmul(rb_ps, onesA, rr2p[(0, b)],
                                           start=True, stop=False)
                          nc.tensor.matmul(rb_ps, onesB, rr2p[(1, b)],
                                           start=False, stop=True)
                          sl = ao_sb[:, eo_hp, b * T:(b + 1) * T]
                          with nc.allow_low_precision(reason="ao8 fp8"):
                              nc.vector.tensor_mul(
                                  out=ao8_sb[:, eo_hp, b * T:(b + 1) * T],
                                  in0=sl, in1=rb_ps)
                  pending_norm.clear()

              for nh in range(E // WQK):
                q_sb = qkpool.tile([P, WQK // P, TOK], BF16, name="qs",
                                   tag="qk", bufs=4)
                k_sb = qkpool.tile([P, WQK // P, TOK], BF16, name="ks",
                                   tag="qk", bufs=4)
                wqs = load_strip(wq_d[l], 0, E, nh * WQK, WQK, "wproj", wpool,
                                 dtype=FP8)
                wks = load_strip(wk_d[l], 0, E, nh * WQK, WQK, "wproj", wpool,
                                 dtype=FP8)
                for wt, dst, si, bt_p in (
                        (wqs, q_sb, 0, params.get(("bq", l))),
                        (wks, k_sb, 1, params.get(("bk", l)))):
                    for ni in range(WQK // P):
                        no = nh * (WQK // P) + ni
                        pss = [psum([P, CH], "pss", "psb", 4)
                               for _ in range(NCH)]
                        for k2 in range(EO // 2):
                            for c in range(NCH):
                                nc.tensor.matmul(
                                    pss[c],
                                    wt[:, 2 * k2:2 * k2 + 2,
                                       ni * P:(ni + 1) * P],
                                    h_sb[:, 2 * k2:2 * k2 + 2, bass.ts(c, CH)],
                                    start=(k2 == 0), stop=(k2 == EO // 2 - 1),
                                    perf_mode=mybir.MatmulPerfMode.DoubleRow)
                        for c in range(NCH):
                            if bt_p is None:
                                nc.vector.tensor_scalar_mul(
                                    dst[:, ni, bass.ts(c, CH)], pss[c],
                                    scl_sb[:, 6 * l + si:6 * l + si + 1])
                            else:
                                nc.scalar.activation(
                                    out=dst[:, ni, bass.ts(c, CH)], in_=pss[c],
                                    func=AF.Identity, bias=bt_p[:, no:no + 1],
                                    scale=scl_sb[:, 6 * l + si:6 * l + si + 1])
                hs0 = nh * WQK // HD
                hs1 = (nh + 1) * WQK // HD
                rr2s = {}
                if nh > 0:
                    flush_norms()
                for h in range(hs0, hs0 if cfg.get("noattn") else hs1):
                  po = (h % HPT) * HD
                  eo_h = h // HPT
                  ni_h = eo_h - nh * (WQK // P)
                  # bias tiles bt[p, tk, tq] = exp(bias[tq, tk*P+p]) (transposed)
                  bt = bpool.tile([P, TQ, T], BF16, name="btile", tag="btile")
                  with tc.high_priority():
                      nc.gpsimd.dma_start(
                          out=bt, in_=bm_d[l, h].rearrange("(tk p) t -> p tk t",
                                                           p=P))
                  if h % 2 == 0:
                      rr2s = {(par, b): rowpool.tile([1, T], BF16, name="rr2",
                                                     tag="rr2", bufs=6)
                              for par in range(2) for b in range(BL)}
                      pending_norm.append((eo_h, rr2s))
                  pts = ptpool.tile([P, TQ, BL, T], FP8, name="pts",
                                    tag="pts")
                  for tk in range(TQ):
                      for b in range(BL):
                          sps = psum([P, T], "sps", "sps", 2)
                          nc.tensor.matmul(
                              sps,
                              k_sb[po:po + HD, ni_h,
                                   b * T + tk * P: b * T + (tk + 1) * P],
                              q_sb[po:po + HD, ni_h, b * T: (b + 1) * T],
                              start=True, stop=True)
                          # pts = 8*exp(s) * exp(bias): exp on ACT (bf16),
                          # bias multiply on Pool (PE stays out of it)
                          ptm = ptpool.tile([P, T], BF16, name="ptm",
                                            tag="ptm", bufs=3)
                          nc.scalar.activation(out=ptm, in_=sps,
                                               func=AF.Exp, bias=ln16_c)
                          with nc.allow_low_precision(reason="pts fp8"):
                              nc.gpsimd.tensor_mul(out=pts[:, tk, b],
                                                   in0=ptm, in1=bt[:, tk, :])
                  for b in range(BL):
                      ops = psum([HD + 1, T], "ops", "ops", 2)
                      for t2 in range(TQ // 2):
                          nc.tensor.matmul(
                              ops,
                              v_sb[:, b * TQ + 2 * t2:b * TQ + 2 * t2 + 2,
                                   h, :],
                              pts[:, 2 * t2:2 * t2 + 2, b, :],
                              start=(t2 == 0), stop=(t2 == TQ // 2 - 1),
                              perf_mode=mybir.MatmulPerfMode.DoubleRow)
                      nc.vector.tensor_copy(
                          out=ao_sb[po:po + HD, eo_h, b * T:(b + 1) * T],
                          in_=ops[:HD, :])
                      with nc.allow_low_precision(reason="recip rows bf16"):
                          nc.vector.reciprocal(
                              out=rr2s[(h % 2, b)], in_=ops[HD:HD + 1, :])


              flush_norms()

              # out projection + residual (in place)
              bo_t = params.get(("bo", l))

              def o_evict(pst, no, c):
                  csl = bass.ts(c, CH)
                  nc.vector.scalar_tensor_tensor(
                      out=x_sb[:, no, csl], in0=pst,
                      scalar=scl_sb[:, 6 * l + 3:6 * l + 4],
                      in1=x_sb[:, no, csl], op0=ALU.mult, op1=ALU.add)
              proj_fm(ao8_sb, wo_d[l], o_evict, dr=True)

              h2_sb = layernorm(params.get(("ln2_g", l)),
                                params.get(("ln2_b", l)))

              # MLP per token chunk (bf16: fp8 anywhere in the MLP costs
              # ~2.1% output error per quantization point -- over budget)
              b1_t = params.get(("b1", l))
              b2_t = params.get(("b2", l))
              for c in range(NCH):
                  csl = bass.ts(c, CH)
                  hid_sb = vhpool.tile([P, FO, CH], BF16, name="hid",
                                       tag="vh")
                  fblk = min(4, FO)
                  for fb in range(FO // fblk):
                      w1c = load_strip(w1_d[l], 0, E, fb * fblk * P,
                                       fblk * P, "w1c", wpool, tag="wproj")
                      for ni in range(fblk):
                          fo = fb * fblk + ni
                          pst = psum([P, CH], "pss", "psb", 4)
                          for ko in range(EO):
                              nc.tensor.matmul(
                                  pst, w1c[:, ko, ni * P:(ni + 1) * P],
                                  h2_sb[:, ko, csl],
                                  start=(ko == 0), stop=(ko == EO - 1))
                          nc.scalar.activation(
                              out=hid_sb[:, fo, :], in_=pst,
                              func=(AF.Tanh if cfg.get("act") == "tanh"
                                    else AF.Gelu),
                              bias=(0.0 if b1_t is None
                                    else b1_t[:, fo:fo + 1]))
                  nblk = min(2, EO)
                  kh_n = 2 if FO % 2 == 0 else 1
                  for nb in range(EO // nblk):
                    w2cs = [load_strip(w2_d[l], kh * (FF // kh_n), FF // kh_n,
                                       nb * nblk * P, nblk * P, "w2c", w2pool)
                            for kh in range(kh_n)]
                    for no_i in range(nblk):
                      no = nb * nblk + no_i
                      pst = psum([P, CH], "pss", "psb", 4)
                      for kh in range(kh_n):
                          for ko in range(FO // kh_n):
                              nc.tensor.matmul(
                                  pst, w2cs[kh][:, ko, no_i * P:(no_i + 1) * P],
                                  hid_sb[:, kh * (FO // kh_n) + ko, :],
                                  start=(kh == 0 and ko == 0),
                                  stop=(kh == kh_n - 1 and
                                        ko == FO // kh_n - 1))
                      if b2_t is None:
                          nc.vector.tensor_add(out=x_sb[:, no, csl], in0=pst,
                                               in1=x_sb[:, no, csl])
                      else:
                          nc.vector.scalar_tensor_tensor(
                              out=x_sb[:, no, csl], in0=pst,
                              scalar=b2_t[:, no:no + 1], in1=x_sb[:, no, csl],
                              op0=ALU.add, op1=ALU.add)

        nc.sync.dma_start(out=out_d.rearrange("(o p) t -> p o t", p=P),
                          in_=x_sb)

    nc.finalize()
    return nc


def host_prep(inputs, cfg):
    BL, T, E, H, FF, L = cfg["BL"], cfg["T"], cfg["E"], cfg["H"], cfg["FF"], cfg["L"]
    HD = E // H
    bf = ml_dtypes.bfloat16
    f32 = np.float32
    inp = {k: np.asarray(v, dtype=np.float32) for k, v in inputs.items()}

    f8 = ml_dtypes.float8_e4m3fn
    shared = {
        "w1": inp["w1"].astype(bf),
        "w2": inp["w2"].astype(bf),
    }
    scl = np.zeros((L, 6), np.float32)
    for nm, si, arr in (("wq", 0, inp["wq"] * (HD ** -0.5)), ("wk", 1, inp["wk"]),
                        ("wv", 2, inp["wv"]), ("wo", 3, inp["wo"])):
        q = np.empty_like(arr, dtype=f8)
        for l in range(L):
            sw = 224.0 / max(np.abs(arr[l]).max(), 1e-30)
            q[l] = (arr[l] * sw).astype(f8)
            # eviction descale: q,k,o fold 1/32 (h8 = 32*h); v folds 32/32
            scl[l, si] = (1.0 / (32.0 * sw)) if si != 2 else (1.0 / sw)
        shared[nm] = q
    shared["scl"] = np.broadcast_to(
        scl.reshape(1, L * 6), (128, L * 6)).copy()
    coords = np.arange(T)
    rel = (coords[:, None] - coords[None, :] + (T - 1)).astype(np.float64)
    bias_all = np.empty((L, H, T, T), dtype=bf)
    for l in range(L):
        off = np.tanh(np.float64(inp["offset"][l, 0])) * MAX_OFFSET
        adj = np.clip(rel + off, 0.0, 2.0 * T - 2.0)
        lo = np.floor(adj).astype(np.int64)
        hi = np.ceil(adj).astype(np.int64)
        w = (adj - lo)[..., None].astype(f32)
        tab = inp["bias_table"][l]
        bm = tab[lo] * (1.0 - w) + tab[hi] * w
        # transposed layout: biasmat[l, h, key_pos, query_pos] = exp(bias)
        # (bias folded into the post-exp multiply on Pool)
        bias_all[l] = np.exp(bm.transpose(2, 1, 0)).astype(bf)
    shared["biasmat"] = bias_all

    flags = set()
    for nm, arr in [("bq", (inp["bq"] * (HD ** -0.5)).astype(f32)),
                    ("bk", inp["bk"]), ("bv", inp["bv"]), ("bo", inp["bo"]),
                    ("b1", inp["b1"]), ("b2", inp["b2"])]:
        if np.any(arr):
            flags.add(nm)
            shared[nm] = np.ascontiguousarray(arr, dtype=f32)
    for pre in ("ln1", "ln2"):
        if np.any(inp[f"{pre}_g"] != 1.0) or np.any(inp[f"{pre}_b"]):
            flags.add(pre)
            shared[f"{pre}_g"] = inp[f"{pre}_g"].astype(f32)
            shared[f"{pre}_b"] = inp[f"{pre}_b"].astype(f32)

    per_core_x = []
    for c in range(N_CORES):
        xs = inp["x"][c * BL:(c + 1) * BL]
        per_core_x.append(np.ascontiguousarray(
            xs.transpose(2, 0, 1).reshape(E, BL * T)))
    return shared, per_core_x, frozenset(flags)


_CACHE = {}


def kernel(**inputs) -> np.ndarray:
    cfg = FULL
    BL, T, E = cfg["BL"], cfg["T"], cfg["E"]
    shared, per_core_x, flags = host_prep(inputs, cfg)
    key = ("full", flags)
    if key not in _CACHE:
        _CACHE[key] = build_nc(cfg, flags)
    nc = _CACHE[key]
    in_maps = [{"x_fm": per_core_x[c], **shared} for c in range(N_CORES)]
    res = run_bass_kernel_spmd(nc, in_maps, core_ids=list(range(N_CORES)))
    out = np.empty((N_CORES * BL, T, E), np.float32)
    for c in range(N_CORES):
        ofm = res.results[c]["out_fm"]
        out[c * BL:(c + 1) * BL] = ofm.reshape(E, BL, T).transpose(1, 2, 0)
    return out

